# revision 11
# baseline (speedup 1.0000x reference)
"""Embedding lookup kernel for Trainium2 (8 NeuronCores, SPMD).

Strategy: token-parallel gather (an embedding lookup IS a row gather:
out[b, s, :] = weight[x[b, s], :]).

- Flatten x [2, 4096] -> [8192] tokens; each of the 8 cores handles 1024
  contiguous tokens. Each core gets the FULL weight table in its DRAM.
- Per core (raw Bacc program, no Tile framework overhead; the Bass entry
  all-engine barrier is skipped — every cross-engine dependency below is
  ordered by an explicit semaphore, so each engine only needs its own
  program order):
    1. One HWDGE DMA loads the 1024 indices as [128, 8] int32 into SBUF
       (partition p holds tokens p*8 .. p*8+7). Completion latency is
       HBM-round-trip-bound (~2.4us), independent of size — splitting the
       load was measured to change nothing.
    2. While that DMA's ~2us HBM-read + completion latency elapses, a dummy
       warmup indirect DMA (indices from a memset-zero tile) runs on the
       Pool engine so the first real gather executes at steady-state cost.
    3. 8 SWDGE indirect DMAs (one per token column j) gather 128 rows each
       (one index per partition — a hard HW limit) into an SBUF tile column
       [128, 128] f32.  SWDGE descriptor generation (~1.1us/op on Q7 pair
       0, serial) is the dominant cost; the 512 KiB of gather traffic
       drains under it.
    4. As each gather's completion semaphore fires, an HWDGE DMA writes
       that column back to DRAM out[:, j*128:(j+1)*128], overlapping the
       remaining gathers. No final completion wait: the NEFF epilogue's
       engine drains already block until the HWDGE queues are empty
       (verified bit-exact on HW).
- out [128, 1024] f32 reshapes host-side to [1024, 128] (token p*8+j at
  partition p, col-block j). Host concatenates the 8 per-core outputs.

No collectives. Measured ~23.4us exec (neuron-profile), bit-exact vs the
one-hot matmul reference.
"""

import contextlib

import numpy as np

import concourse.bass as bass
from concourse import bacc, mybir
from concourse.bass_utils import run_bass_kernel_spmd

N_CORES = 8
B, S = 2, 4096
VOCAB, DIM = 32000, 128
P = 128
TOKENS = B * S                      # 8192
TPC = TOKENS // N_CORES             # 1024 tokens per core
TPP = TPC // P                      # 8 tokens per partition


def build_nc():
    # Skip the Bass-constructor entry barrier (gates the first DMA behind
    # all engines' init); restore the method right after construction.
    orig_barrier = bass.Bass.all_engine_barrier
    bass.Bass.all_engine_barrier = lambda self, *a, **k: None
    try:
        nc = bacc.Bacc(None, target_bir_lowering=False)
    finally:
        bass.Bass.all_engine_barrier = orig_barrier

    x = nc.dram_tensor("x", [P, TPP], mybir.dt.int32, kind="ExternalInput")
    w = nc.dram_tensor("weight", [VOCAB, DIM], mybir.dt.float32, kind="ExternalInput")
    out = nc.dram_tensor("out", [P, TPC], mybir.dt.float32, kind="ExternalOutput")

    with contextlib.ExitStack() as ctx:
        idx_tile = ctx.enter_context(
            nc.sbuf_tensor("idx_tile", [P, TPP], mybir.dt.int32)
        )
        g = ctx.enter_context(nc.sbuf_tensor("g", [P, TPC], mybir.dt.float32))
        dummy_idx = ctx.enter_context(
            nc.sbuf_tensor("dummy_idx", [P, 1], mybir.dt.int32)
        )
        scratch = ctx.enter_context(
            nc.sbuf_tensor("scratch", [P, DIM], mybir.dt.float32)
        )
        s_idx = ctx.enter_context(nc.semaphore("s_idx"))
        s_out = ctx.enter_context(nc.semaphore("s_out"))
        s_warm = ctx.enter_context(nc.semaphore("s_warm"))
        s_ms = ctx.enter_context(nc.semaphore("s_ms"))
        s_gs = [ctx.enter_context(nc.semaphore(f"s_g{j}")) for j in range(TPP)]

        nc.sync.dma_start(idx_tile[:], x[:]).then_inc(s_idx, 16)

        # Warmup gather, hidden inside the idx-DMA latency window.
        nc.gpsimd.memset(dummy_idx[:], 0).then_inc(s_ms, 1)
        nc.gpsimd.wait_ge(s_ms, 1)
        nc.gpsimd.indirect_dma_start(
            out=scratch[:],
            out_offset=None,
            in_=w[:],
            in_offset=bass.IndirectOffsetOnAxis(ap=dummy_idx[:], axis=0),
        ).then_inc(s_warm, 16)

        nc.gpsimd.wait_ge(s_idx, 16)
        for j in range(TPP):
            nc.gpsimd.indirect_dma_start(
                out=g[:, j * DIM : (j + 1) * DIM],
                out_offset=None,
                in_=w[:],
                in_offset=bass.IndirectOffsetOnAxis(ap=idx_tile[:, j : j + 1], axis=0),
            ).then_inc(s_gs[j], 16)
        for j in range(TPP):
            nc.sync.wait_ge(s_gs[j], 16)
            nc.sync.dma_start(
                out[:, j * DIM : (j + 1) * DIM], g[:, j * DIM : (j + 1) * DIM]
            ).then_inc(s_out, 16)
    nc.compile()
    return nc


_NC_CACHE = None


def kernel(x: np.ndarray, weight: np.ndarray, **run_kwargs):
    global _NC_CACHE
    if _NC_CACHE is None:
        _NC_CACHE = build_nc()
    nc = _NC_CACHE

    x_flat = np.asarray(x).reshape(-1).astype(np.int32)
    w = np.ascontiguousarray(np.asarray(weight, dtype=np.float32))

    in_maps = [
        {
            "x": np.ascontiguousarray(x_flat[c * TPC : (c + 1) * TPC].reshape(P, TPP)),
            "weight": w,
        }
        for c in range(N_CORES)
    ]
    res = run_bass_kernel_spmd(nc, in_maps, core_ids=list(range(N_CORES)), **run_kwargs)
    # out [128, 1024] -> [1024, 128]: token p*TPP+j lives at [p, j*DIM:(j+1)*DIM]
    parts = [res.results[c]["out"].reshape(TPC, DIM) for c in range(N_CORES)]
    full = np.concatenate(parts, axis=0).reshape(B, S, DIM)
    if run_kwargs:
        return full, res
    return full
